# revision 24
# baseline (speedup 1.0000x reference)
"""Trainium2 Bass kernel for nn_MGEmbedder (gnn_message_passing).

Computation (see reference):
    embs = mg_emb[var_indices]                  # [1,4,4,12288,16]
    nb   = embs[:, :, :, adjc, :]               # [1,4,4,12288,9,16]
    x    = nb.reshape(..., 144)
    y    = x @ W + b                            # [1,4,4,12288,1024]
    out  = y.reshape(1,4,4,12288*16,64)

Strategy:
  - Shard the grid/patch dimension (12288) across 8 NeuronCores (1536 each).
  - Host pre-pass: select the 16 used (v,t) tables via var_indices and
    re-layout to emb_r[g, vt*16+c] (bf16) so one gather index moves a
    512B-contiguous element (all 16 (v,t) slices for one neighbor).
  - Device: indirect-DMA gather of neighborhoods -> PE transpose to get
    features on partitions -> bf16 matmul with W (fp32 PSUM accum)
    -> PSUM->SBUF copy (DVE/ACT alternating) -> DMA out fp32.
"""

import numpy as np
import ml_dtypes

import concourse.bass as bass
import concourse.bacc as bacc
import concourse.mybir as mybir
import concourse.tile as tile
from concourse.bass_utils import run_bass_kernel_spmd
from concourse.masks import make_identity

# -- problem dims (hardcoded per spec) --
NG, NH, C = 12288, 9, 16
B, V, T = 1, 4, 4
VT = B * V * T            # 16 flattened (b,v,t) slices
F = NH * C                # 144
UP, E = 16, 64
O = UP * E                # 1024
NCORES = 8
PC = NG // NCORES         # 1536 patches per core
PT = 128                  # patch tile (partition dim)
NPT = PC // PT            # 12 patch tiles per core
VG = 4                    # vts per transpose-psum group
NID = PT * NH // 16       # idx columns per ptile for dma_gather (72)

BF16 = mybir.dt.bfloat16
F32 = mybir.dt.float32
I32 = mybir.dt.int32


def build_nc(npt=NPT, loops=1):
    # Bacc (not raw Bass): its compile() runs generate_event_semaphores,
    # which splits multi-wait instructions into EventSemaphore carriers —
    # TRN2 instructions only have one HW wait slot.
    nc = bacc.Bacc("TRN2")
    emb = nc.dram_tensor("emb", [NG, VT * C], BF16, kind="ExternalInput")
    # gather indices: idx[p, t*9+j] = adjc[core_base + t*128 + p, j].
    # HW indirect_dma_start semantics: ONE index per output partition row,
    # each moving the full free-size contiguously from in_[idx[p]].
    idx = nc.dram_tensor("idx", [PT, NPT * NH], I32, kind="ExternalInput")
    w = nc.dram_tensor("w", [F, O], BF16, kind="ExternalInput")
    y = nc.dram_tensor("y", [VT, PC, O], F32, kind="ExternalOutput")

    with tile.TileContext(nc) as tc:
        with (
            tc.tile_pool(name="const", bufs=1) as constp,
            tc.tile_pool(name="xg", bufs=npt) as xgp,
            tc.tile_pool(name="xs", bufs=2) as xsp,
            tc.tile_pool(name="xt", bufs=3) as xtp,
            tc.tile_pool(name="outb", bufs=4) as outp,
            tc.tile_pool(name="pst", bufs=2, space="PSUM") as pstp,
            tc.tile_pool(name="pso", bufs=2, space="PSUM") as psop,
        ):
            w0 = constp.tile([128, O], BF16)
            nc.sync.dma_start(out=w0[:], in_=w[0:128, :])
            w1 = constp.tile([16, O], BF16)
            nc.sync.dma_start(out=w1[:], in_=w[128:F, :])
            idx_sb = constp.tile([PT, NPT * NH], I32)
            nc.sync.dma_start(out=idx_sb[:], in_=idx[:])
            ident = constp.tile([128, 128], BF16)
            make_identity(nc, ident[:])

            if loops > 1:
                # timing mode: repeat the body on-device to amortize the
                # ~250ms axon dispatch overhead out of the measurement
                loop_cm = tc.For_i(0, loops, 1)
                loop_cm.__enter__()

            flip = 0
            for t in range(npt):
                # gather all 16 (v,t) slices for the 9 neighbors of 128 patches
                # xa[p, j*256 + vt*16 + c] = emb_r[adjc[gp, j], vt*16 + c]
                xa = xgp.tile([PT, NH * VT * C], BF16)  # [128, 2304]
                for j in range(NH):
                    nc.gpsimd.indirect_dma_start(
                        out=xa[:, j * VT * C:(j + 1) * VT * C],
                        out_offset=None,
                        in_=emb[:],
                        in_offset=bass.IndirectOffsetOnAxis(
                            ap=idx_sb[:, t * NH + j:t * NH + j + 1], axis=0
                        ),
                    )
                # shuffle [p, (j, vt, c)] -> [p, (vt, j, c)] so transpose
                # inputs are contiguous (walrus: stationary AP must be 2D)
                xa2 = xsp.tile([PT, VT * F], BF16)  # [128, 2304]
                nc.vector.tensor_copy(
                    out=xa2[:],
                    in_=xa[:].rearrange("p (j v c) -> p v j c", j=NH, v=VT),
                )

                for g in range(VT // VG):
                    # PE transposes for VG vts into one psum bank (bf16)
                    pt_ = pstp.tile([128, VG * 256], BF16)
                    for q in range(VG):
                        vt = g * VG + q
                        # first 8 neighbors: [128p, (8j,16c)] -> [128f, 128p]
                        nc.tensor.transpose(
                            out=pt_[0:128, q * 256:q * 256 + 128],
                            in_=xa2[:, vt * F:vt * F + 128],
                            identity=ident[:],
                        )
                        # 9th neighbor: [128p, 16c] -> [16f, 128p]
                        nc.tensor.transpose(
                            out=pt_[0:16, q * 256 + 128:q * 256 + 256],
                            in_=xa2[:, vt * F + 128:(vt + 1) * F],
                            identity=ident[:],
                        )
                    # copy transposed activations PSUM->SBUF. Separate A/B dest
                    # tiles keep each copy's deps within the PE semaphore
                    # family (ACT instructions only get one HW wait slot).
                    xta = xtp.tile([128, VG * 128], BF16, tag="xta")
                    xtb = xtp.tile([16, VG * 128], BF16, tag="xtb")
                    ptvA = pt_[:].rearrange("p (q x) -> p q x", q=VG)[:, :, 0:128]
                    xtvA = xta[:].rearrange("p (q x) -> p q x", q=VG)
                    ptvB = pt_[0:16, :].rearrange("p (q x) -> p q x", q=VG)[:, :, 128:256]
                    xtvB = xtb[:].rearrange("p (q x) -> p q x", q=VG)
                    # both copies of a group on ONE engine: the PSUM bank
                    # tracker serializes cross-engine access to one bank,
                    # which would add a second wait family.
                    if g % 2 == 0:
                        nc.vector.tensor_copy(out=xtvA, in_=ptvA)
                        nc.vector.tensor_copy(out=xtvB, in_=ptvB)
                    else:
                        nc.scalar.copy(out=xtvA, in_=ptvA)
                        nc.scalar.copy(out=xtvB, in_=ptvB)

                    for q in range(VG):
                        vt = g * VG + q
                        po = psop.tile([128, O], F32)
                        lA = xta[:, q * 128:(q + 1) * 128]   # [128K, 128M]
                        lB = xtb[0:16, q * 128:(q + 1) * 128]  # [16K, 128M]
                        nc.tensor.matmul(po[:, 0:512], lhsT=lA, rhs=w0[:, 0:512],
                                         start=True, stop=False)
                        nc.tensor.matmul(po[:, 512:1024], lhsT=lA, rhs=w0[:, 512:1024],
                                         start=True, stop=False)
                        nc.tensor.matmul(po[:, 0:512], lhsT=lB, rhs=w1[:, 0:512],
                                         start=False, stop=True)
                        nc.tensor.matmul(po[:, 512:1024], lhsT=lB, rhs=w1[:, 512:1024],
                                         start=False, stop=True)
                        # evacuate PSUM, alternating engines per vt
                        ob = outp.tile([128, O], F32)
                        if flip == 0:
                            nc.vector.tensor_copy(out=ob[:], in_=po[:])
                        else:
                            nc.scalar.copy(out=ob[:], in_=po[:])
                        flip ^= 1
                        nc.sync.dma_start(out=y[vt, t * PT:(t + 1) * PT, :], in_=ob[:])
            if loops > 1:
                loop_cm.__exit__(None, None, None)
    nc.compile()
    return nc


_NC_CACHE = {}


def _get_nc(npt=NPT):
    if npt not in _NC_CACHE:
        _NC_CACHE[npt] = build_nc(npt)
    return _NC_CACHE[npt]


def _prep_inputs(mg_emb, var_indices, adjc, W):
    """Host-side re-layout: returns (in_maps, ...) for the 8 cores."""
    emb_sel = mg_emb[var_indices[0]]  # [V, T, NG, C]
    emb_r = np.ascontiguousarray(
        emb_sel.transpose(2, 0, 1, 3).reshape(NG, VT * C)
    ).astype(ml_dtypes.bfloat16)
    w_bf = np.ascontiguousarray(W).astype(ml_dtypes.bfloat16)
    in_maps = []
    for cid in range(NCORES):
        a = adjc[cid * PC:(cid + 1) * PC].astype(np.int32)  # [1536, 9]
        idxc = np.ascontiguousarray(
            a.reshape(NPT, PT, NH).transpose(1, 0, 2).reshape(PT, NPT * NH)
        )
        in_maps.append({"emb": emb_r, "idx": idxc, "w": w_bf})
    return in_maps


def kernel(mg_emb, var_indices, adjc, W, b, trace=False, tmpdir=None):
    mg_emb = np.asarray(mg_emb, dtype=np.float32)
    var_indices = np.asarray(var_indices).astype(np.int64)
    adjc = np.asarray(adjc).astype(np.int64)
    W = np.asarray(W, dtype=np.float32)
    b = np.asarray(b, dtype=np.float32)
    assert mg_emb.shape == (8, T, NG, C), mg_emb.shape
    assert var_indices.shape == (B, V), var_indices.shape
    assert adjc.shape == (NG, NH), adjc.shape
    assert W.shape == (F, O) and b.shape == (O,)

    in_maps = _prep_inputs(mg_emb, var_indices, adjc, W)
    nc = _get_nc()
    res = run_bass_kernel_spmd(
        nc, in_maps, core_ids=list(range(NCORES)), trace=trace, tmpdir=tmpdir
    )

    y = np.empty((VT, NG, O), dtype=np.float32)
    for cid in range(NCORES):
        y[:, cid * PC:(cid + 1) * PC, :] = res.results[cid]["y"]
    if np.any(b):
        y += b[None, None, :]
    out = y.reshape(V, T, NG, UP, E).reshape(B, V, T, NG * UP, E)
    if trace:
        return out, res
    return out
